# revision 23
# baseline (speedup 1.0000x reference)
"""Trainium2 Bass kernel for nn_LogicityPredictorVis.

The reference returns agg + x @ root + bias with shape [8, 4], which depends
ONLY on batch element 0 of every batched input (node_concepts[0], edge_attr[0],
batch_priorities[0]).  The B=4096 MLP sweep is dead code w.r.t. the output, so
the kernel computes just the batch-0 path.

Sharding: the NODE_CH=2048 contraction (node-MLP layer 3, the NNConv einsum,
and x @ root) is split over the 8 cores (256 channels each).  The small
replicated layers (node-MLP layers 1/2, edge MLP, pr layer 1) run on every
core.  Each core emits a partial [8,4] result; the host sums them.

The graded metric is the TimelineSim device-occupancy time of the (identical)
per-core program.  Bound by: the serial 360 GB/s DMA pipe (~5.2 us of fp16
images), the ~1.9 us issue->transfer pipe-start latency, the ~900 ns DMA
completion-semaphore propagation, the serialized cross-engine hop chain from
the last weight to the output DMA, and ~2.4 us of output-DMA + barrier
epilogue.  Design points:

* Everything DMA'd is float16 (PSUM accumulates fp32); ~1.8 MB/core, 1
  cycle/row matmuls, rel err ~1e-3 vs the 2e-2 gate.
* Eight DMAs, four per HWDGE-issuing engine (SP / Activation), interleaved so
  the serial descriptor-gen chain never starves the transfer pipe and images
  land in consumer order: w1ax, b1row, b64, w1b, w2x, w3, pw2a, pw2b.
* ALL layer biases are K=1 matmuls into the accumulation groups, fed from a
  4-ns one-row image (b1row), so every relu is a single DVE max op and the
  DVE queue (the scarce resource: each op costs ~300 ns with its ack gap)
  carries only: priority compare, 5 max ops, rhs2, 2 prod2 halves, 3 PSUM
  copies.
* The einsum weights pw2 ride last, split in two halves so the first two
  G o-slices and their prod2 multiply run under the last transfer; the final
  accumulation is one [8,4] PSUM group (bias + x@root via eye + xb via the
  complete-graph (1-I) identity + dselp-aggregated messages), one copy, one
  SP dma_start.
"""

import numpy as np

B, N = 4096, 8
C_IMG = 1024
NODE_CH = 2048
EDGE_CH = 3
ACT_CH = 4
E = N * (N - 1)
BBOX_MAX = 1024.0
N_CORES = 8
CS = NODE_CH // N_CORES        # 256 channels per core
C4O = CS * ACT_CH              # 1024 (c,o) pairs per core

_IDX = np.array([[i, j] for i in range(N) for j in range(N) if i != j],
                dtype=np.int32)
SRC = _IDX[:, 0]
DST = _IDX[:, 1]

# ---- packed fp16 input images ----
_W1AX = [("x0T", 64), ("ones128", 1), ("pb1col", 1), ("b1cols", 4),
         ("eb1cols", 2), ("ew2", 128), ("w1a", 2048)]
_W2X = [("w2", 1024), ("rootpb", 16)]
_B64 = [
    ("ew3", 21), ("dselp", 8), ("pw1r", 128), ("maskblk", 56),
    ("ew1", 256), ("attrT", 8), ("oneminusI", 8), ("eye8", 8),
    ("p0row", 8), ("ones8", 8), ("eb3row", 21), ("biasrow4", 4),
    ("eb2col", 1), ("pad", 56),
]
_B64_PARTS = {
    "ew3": 64, "dselp": 56, "pw1r": 28, "maskblk": 28, "ew1": 8,
    "attrT": 8, "oneminusI": 8, "eye8": 8, "p0row": 1, "ones8": 1,
    "eb3row": 1, "biasrow4": 1, "eb2col": 64, "pad": 1,
}
# Bias rows + ones rows ride in b64's dead rows (under the <=28-partition
# tensors), at 32-aligned partitions so K=1 matmul operands are legal.
# name -> (row, col, width); each bias matmul pairs with the ones8 copy in
# the same row (matmul requires equal base partitions).
_B64_SLOTS = {
    "b2m0": (32, 29, 128), "b2m1": (32, 157, 128),
    "b3m0": (32, 285, 128), "b3m1": (32, 413, 128),
    "ones8_32": (32, 541, 8),
}


def _offsets(specs):
    offs, off = {}, 0
    for n, c in specs:
        offs[n] = off
        off += c
    return offs, off


_OFF_W1AX, COLS_W1AX = _offsets(_W1AX)
_OFF_W2X, COLS_W2X = _offsets(_W2X)
_OFF_B64, COLS_B64 = _offsets(_B64)

_NC_CACHE = {}


def build_nc():
    """Build the per-core Bass program (identical on all cores)."""
    import concourse.bacc as bacc
    import concourse.mybir as mybir
    import concourse.tile as tile

    fp32 = mybir.dt.float32
    fp16 = mybir.dt.float16
    AF = mybir.ActivationFunctionType
    ALU = mybir.AluOpType

    nc = bacc.Bacc("TRN2", target_bir_lowering=False, debug=False)
    w1ax_d = nc.dram_tensor("w1ax", [128, COLS_W1AX], fp16,
                            kind="ExternalInput")
    w1b_d = nc.dram_tensor("w1b", [128, 2048], fp16, kind="ExternalInput")
    w2x_d = nc.dram_tensor("w2x", [128, COLS_W2X], fp16, kind="ExternalInput")
    w3_d = nc.dram_tensor("w3", [128, 512], fp16, kind="ExternalInput")
    pw2a_d = nc.dram_tensor("pw2a", [128, 512], fp16, kind="ExternalInput")
    pw2b_d = nc.dram_tensor("pw2b", [128, 512], fp16, kind="ExternalInput")
    b64_d = nc.dram_tensor("b64", [64, COLS_B64], fp16, kind="ExternalInput")
    outB_d = nc.dram_tensor("outB", [8, 4], fp32, kind="ExternalOutput")

    with tile.TileContext(nc) as tc:
        with tc.tile_pool(name="sb", bufs=1) as sb, \
             tc.tile_pool(name="ps", bufs=1, space="PSUM") as ps:

            w1ax_sb = sb.tile([128, COLS_W1AX], fp16, tag="w1ax")
            w1b_sb = sb.tile([128, 2048], fp16, tag="w1b")
            w2x_sb = sb.tile([128, COLS_W2X], fp16, tag="w2x")
            w3_sb = sb.tile([128, 512], fp16, tag="w3")
            pw2a_sb = sb.tile([128, 512], fp16, tag="pw2a")
            pw2b_sb = sb.tile([128, 512], fp16, tag="pw2b")
            b64_sb = sb.tile([64, COLS_B64], fp16, tag="b64")

            # ---- DMA issue.  SP: b64, w1b, w2x, w3, pw2a, pw2b (serial,
            # 650 ns apart); Act: w1ax.  HWDGE arrival (= transfer) order:
            # b64, w1ax, w1b, w2x, w3, pw2a, pw2b — b64 lands FIRST so the
            # edge-chain ladder (whose g2T stationary-load otherwise
            # head-blocks the in-order PE stream right when the w1b-gated
            # L1 m23 matmuls become ready) completes early.
            nc.sync.dma_start(b64_sb[:], b64_d[:])
            nc.sync.dma_start(w1b_sb[:], w1b_d[:])
            nc.sync.dma_start(w2x_sb[:], w2x_d[:])
            nc.sync.dma_start(w3_sb[:], w3_d[:])
            nc.sync.dma_start(pw2a_sb[:], pw2a_d[:])
            nc.sync.dma_start(pw2b_sb[:], pw2b_d[:])
            nc.scalar.dma_start(w1ax_sb[:], w1ax_d[:])

            def vax(name):
                off = _OFF_W1AX[name]
                cc = dict(_W1AX)[name]
                return w1ax_sb[:, off:off + cc]

            def v64(name):
                off = _OFF_B64[name]
                cc = dict(_B64)[name]
                return b64_sb[0:_B64_PARTS[name], off:off + cc]

            def vb(name):
                r, c, w = _B64_SLOTS[name]
                return b64_sb[r:r + 1, c:c + w]

            def bias_mm(out_view, slot):
                """K=1 group-stop bias matmul from the b64 row-32 slots."""
                nc.tensor.matmul(out_view, vb(slot), vb("ones8_32"),
                                 start=False, stop=True,
                                 skip_group_check=True)

            x0T_v = vax("x0T").rearrange("p (q n) -> p q n", q=8)
            w1a_v = vax("w1a").rearrange("p (m q k) -> p m q k", m=2, q=8)
            w1b_v = w1b_sb[:].rearrange("p (m q k) -> p m q k", m=2, q=8)
            ew2_v = vax("ew2").rearrange("p (q m) -> p q m", q=2)
            ones128_v = vax("ones128")
            w2_v = w2x_sb[:, 0:1024].rearrange("p (q m) -> p q m", q=4)
            rootpb_v = w2x_sb[:, _OFF_W2X["rootpb"]:].rearrange(
                "p (q m) -> p q m", q=2)
            w3_v = w3_sb[:].rearrange("p (q m) -> p q m", q=2)
            pw2a_v = pw2a_sb[:].rearrange("p (o q h) -> p o q h", o=2, q=2)
            pw2b_v = pw2b_sb[:].rearrange("p (o q h) -> p o q h", o=2, q=2)
            ew3_v, dselp_v, pw1r_v = v64("ew3"), v64("dselp"), v64("pw1r")
            maskblk_v, ew1_v, attrT_v = v64("maskblk"), v64("ew1"), v64("attrT")
            oneminusI_v, eye8_v = v64("oneminusI"), v64("eye8")
            p0row_v, ones8_v = v64("p0row"), v64("ones8")

            # fp32 scalar columns for the fused (add bias, max 0) relus
            # whose bias is per-partition (g2T, tT): DVE scalar port is fp32.
            pb1f_sb = sb.tile([128, 1], fp32, tag="pb1f")
            nc.vector.tensor_copy(pb1f_sb[:], vax("pb1col"))
            eb2f_sb = sb.tile([64, 1], fp32, tag="eb2f")
            nc.vector.tensor_copy(eb2f_sb[:], v64("eb2col"))

            # ============ compute, in dataflow order ========================
            # L1 m=0,1 (gated on w1ax; bias mm per group from b1row)
            p_h1 = ps.tile([128, 4, N], fp32, tag="ps_n", bufs=2)
            for m in range(2):
                for q in range(8):
                    nc.tensor.matmul(p_h1[:, m, :], w1a_v[:, m, q, :],
                                     x0T_v[:, q, :], start=(q == 0),
                                     stop=(q == 7), skip_group_check=True)
            # ---- edge chain (gated on b64 + b1row) ----
            p_pp = ps.tile([8, 2, 8], fp32, tag="ps_e", bufs=1)
            nc.tensor.matmul(p_pp[:, 0, :], p0row_v, ones8_v, start=True,
                             stop=True, skip_group_check=True)
            nc.tensor.matmul(p_pp[:, 1, :], ones8_v, p0row_v, start=True,
                             stop=True, skip_group_check=True)
            q4_sb = sb.tile([8, 7, 4], fp16, tag="q4")
            pc_sb = sb.tile([8, 8], fp16, tag="pc")
            nc.vector.tensor_copy(pc_sb[:], p_pp[:, 0, :])
            nc.vector.tensor_tensor(q4_sb[:, :, 3], pc_sb[:, 0:7],
                                    p_pp[:, 1, 0:7], op=ALU.is_gt)
            p_g1 = ps.tile([128, 2, N], fp32, tag="ps_e", bufs=1)
            for m in range(2):
                nc.tensor.matmul(p_g1[:, m, :],
                                 ew1_v[:, m * 128:(m + 1) * 128],
                                 attrT_v, start=True, stop=True,
                                 skip_group_check=True)
            g1T_sb = sb.tile([128, 2, N], fp16, tag="g1T")
            nc.vector.tensor_tensor(g1T_sb[:], p_g1[:],
                                    vax("eb1cols").unsqueeze(2)
                                    .broadcast_to([128, 2, N]),
                                    op=ALU.add)
            nc.vector.tensor_scalar(g1T_sb[:], g1T_sb[:], 0.0, None,
                                    op0=ALU.max)
            p_g2 = ps.tile([64, N], fp32, tag="ps_e", bufs=1)
            for q in range(2):
                nc.tensor.matmul(p_g2[:], ew2_v[:, q, :], g1T_sb[:, q, :],
                                 start=(q == 0), stop=(q == 1),
                                 skip_group_check=True)
            g2T_sb = sb.tile([64, N], fp16, tag="g2T")
            nc.vector.tensor_scalar(g2T_sb[:], p_g2[:], eb2f_sb[:], 0.0,
                                    op0=ALU.add, op1=ALU.max)
            p_ea = ps.tile([8, 21], fp32, tag="ps_e", bufs=1)
            nc.tensor.matmul(p_ea[:], g2T_sb[:], ew3_v, start=True,
                             stop=False, skip_group_check=True)
            nc.tensor.matmul(p_ea[:], ones8_v, v64("eb3row"), start=False,
                             stop=True, skip_group_check=True)
            nc.scalar.activation(q4_sb[:, :, 0:3],
                                 p_ea[:].rearrange("i (j c) -> i j c", c=3),
                                 AF.Sigmoid)
            # ---- L1 m=2,3 (gated on w1b), single wide max ----
            for m in range(2):
                for q in range(8):
                    nc.tensor.matmul(p_h1[:, m + 2, :], w1b_v[:, m, q, :],
                                     x0T_v[:, q, :], start=(q == 0),
                                     stop=(q == 7), skip_group_check=True)
            h1T_sb = sb.tile([128, 4, N], fp16, tag="h1T")
            nc.vector.tensor_tensor(h1T_sb[:], p_h1[:],
                                    vax("b1cols").unsqueeze(2)
                                    .broadcast_to([128, 4, N]),
                                    op=ALU.add)
            nc.vector.tensor_scalar(h1T_sb[:], h1T_sb[:], 0.0, None,
                                    op0=ALU.max)
            # ---- edge tail: transpose, rhs2 (PSUM-direct), t ----
            p_q4T = ps.tile([28, 8], fp16, tag="ps_e", bufs=1)
            nc.tensor.transpose(p_q4T[:],
                                q4_sb[:].rearrange("i j c -> i (j c)"),
                                eye8_v)
            rhs2_sb = sb.tile([28, E], fp16, tag="rhs2")
            nc.vector.tensor_tensor(
                rhs2_sb[:].rearrange("p (j i) -> p j i", i=8),
                p_q4T[:].unsqueeze(1).broadcast_to([28, 7, N]),
                maskblk_v.rearrange("p (j i) -> p j i", i=8),
                op=ALU.mult)
            p_t = ps.tile([128, E], fp32, tag="ps_G", bufs=2)
            nc.tensor.matmul(p_t[:], pw1r_v, rhs2_sb[:], start=True,
                             stop=True)
            tT_sb = sb.tile([128, E], fp16, tag="tT")
            nc.vector.tensor_scalar(tT_sb[:], p_t[:], pb1f_sb[:], 0.0,
                                    op0=ALU.add, op1=ALU.max)
            # ---- L2 (gated on w2x), single max ----
            p_h2 = ps.tile([128, 2, N], fp32, tag="ps_n", bufs=2)
            for m in range(2):
                for q in range(4):
                    nc.tensor.matmul(p_h2[:, m, :],
                                     w2_v[:, q, m * 128:(m + 1) * 128],
                                     h1T_sb[:, q, :], start=(q == 0),
                                     stop=False, skip_group_check=True)
                bias_mm(p_h2[:, m, :], "b2m%d" % m)
            h2T_sb = sb.tile([128, 2, N], fp16, tag="h2T")
            nc.vector.tensor_scalar(h2T_sb[:], p_h2[:], 0.0, None,
                                    op0=ALU.max)
            # ---- L3 (gated on w3) + bias mm, single sigmoid ----
            p_x = ps.tile([128, 2, N], fp32, tag="ps_n", bufs=2)
            for m in range(2):
                for q in range(2):
                    nc.tensor.matmul(p_x[:, m, :],
                                     w3_v[:, q, m * 128:(m + 1) * 128],
                                     h2T_sb[:, q, :], start=(q == 0),
                                     stop=False, skip_group_check=True)
                bias_mm(p_x[:, m, :], "b3m%d" % m)
            xT_sb = sb.tile([128, 2, N], fp16, tag="xT")
            nc.scalar.activation(xT_sb[:], p_x[:], AF.Sigmoid)
            # ---- x @ [root | pb2]; G + prod2 in pw2a / pw2b halves ----
            p_o2 = ps.tile([8, 8], fp32, tag="ps_g", bufs=3)
            for q in range(2):
                nc.tensor.matmul(p_o2[:], xT_sb[:, q, :], rootpb_v[:, q, :],
                                 start=(q == 0), stop=(q == 1),
                                 skip_group_check=True)
            p_G = ps.tile([128, 4, N], fp32, tag="ps_G", bufs=2)
            prod2_sb = sb.tile([128, 4, 7, N], fp16, tag="prod2")
            for o in range(2):
                for q in range(2):
                    nc.tensor.matmul(p_G[:, o, :], pw2a_v[:, o, q, :],
                                     xT_sb[:, q, :], start=(q == 0),
                                     stop=(q == 1), skip_group_check=True)
            nc.vector.tensor_tensor(
                prod2_sb[:, 0:2, :, :],
                tT_sb[:].rearrange("p (j i) -> p j i", i=8).unsqueeze(1)
                        .broadcast_to([128, 2, 7, N]),
                p_G[:, 0:2, :].unsqueeze(2).broadcast_to([128, 2, 7, N]),
                op=ALU.mult)
            o2_sb = sb.tile([8, 8], fp16, tag="o2")
            nc.vector.tensor_copy(o2_sb[:], p_o2[:])
            for o in range(2):
                for q in range(2):
                    nc.tensor.matmul(p_G[:, o + 2, :], pw2b_v[:, o, q, :],
                                     xT_sb[:, q, :], start=(q == 0),
                                     stop=(q == 1), skip_group_check=True)
            nc.vector.tensor_tensor(
                prod2_sb[:, 2:4, :, :],
                tT_sb[:].rearrange("p (j i) -> p j i", i=8).unsqueeze(1)
                        .broadcast_to([128, 2, 7, N]),
                p_G[:, 2:4, :].unsqueeze(2).broadcast_to([128, 2, 7, N]),
                op=ALU.mult)
            # ---- final accumulation group + per-edge sums ----
            p_s4 = ps.tile([56, 4], fp32, tag="ps_g", bufs=3)
            for o in range(4):
                nc.tensor.matmul(p_s4[:, o:o + 1],
                                 prod2_sb[:, o, :, :].rearrange(
                                     "p j i -> p (j i)"),
                                 ones128_v, start=True, stop=True,
                                 skip_group_check=True)
            s4_sb = sb.tile([56, 4], fp16, tag="s4")
            nc.vector.tensor_copy(s4_sb[:], p_s4[:])
            p_o3 = ps.tile([8, 4], fp32, tag="ps_g", bufs=3)
            nc.tensor.matmul(p_o3[:], ones8_v, v64("biasrow4"),
                             start=True, stop=False, skip_group_check=True)
            nc.tensor.matmul(p_o3[:], eye8_v, o2_sb[:, 0:4], start=False,
                             stop=False, skip_group_check=True)
            nc.tensor.matmul(p_o3[:], oneminusI_v, o2_sb[:, 4:8],
                             start=False, stop=False, skip_group_check=True)
            nc.tensor.matmul(p_o3[:], dselp_v, s4_sb[:], start=False,
                             stop=True, skip_group_check=True)
            o3_sb = sb.tile([8, 4], fp32, tag="o3")
            nc.vector.tensor_copy(o3_sb[:], p_o3[:])
            nc.sync.dma_start(outB_d[:], o3_sb[:])

    nc.compile()
    return nc


def _chunked(x, q):
    """[q*128, m] -> [128, q*m] image (partition p holds chunk-major rows)."""
    q128, m = x.shape
    assert q128 == q * 128
    return x.reshape(q, 128, m).transpose(1, 0, 2).reshape(128, q * m)


def make_in_maps(inputs):
    """Host-side sharding: build the per-core packed fp16 images (numpy)."""
    f16 = np.float16
    f32 = np.float32

    def a(x):
        return np.ascontiguousarray(np.asarray(x, dtype=f32))

    roi = a(inputs["roi_features"][0])
    bbox = a(inputs["batch_bboxes"][0])
    dirs = a(inputs["batch_directions"][0])
    p0 = a(inputs["batch_priorities"][0])

    w1ax = np.zeros((128, COLS_W1AX), f16)
    w1b = np.zeros((128, 2048), f16)
    w2x = np.zeros((128, COLS_W2X), f16)
    b64 = np.zeros((64, COLS_B64), f16)

    def put(img, offs, widths, name, data):
        off = offs[name]
        cc = widths[name]
        data = np.asarray(data, f16)
        pp = data.shape[0]
        assert data.shape == (pp, cc), (name, data.shape, cc)
        img[0:pp, off:off + cc] = data

    wax, w2xw, b64w = dict(_W1AX), dict(_W2X), dict(_B64)
    w1 = a(inputs["ncp_w1"]).reshape(8, 128, 4, 128)
    w1mqk = np.ascontiguousarray(w1.transpose(1, 2, 0, 3)).reshape(128, 4096)
    put(w1ax, _OFF_W1AX, wax, "w1a", w1mqk[:, 0:2048])
    w1b[:] = w1mqk[:, 2048:4096]
    put(w1ax, _OFF_W1AX, wax, "x0T", _chunked(a(roi.T), 8))
    put(w1ax, _OFF_W1AX, wax, "ones128", np.ones((128, 1), f16))
    put(w1ax, _OFF_W1AX, wax, "ew2", _chunked(a(inputs["ep_w2"]), 2))

    put(w2x, _OFF_W2X, w2xw, "w2", _chunked(a(inputs["ncp_w2"]), 4))

    put(b64, _OFF_B64, b64w, "ew3", a(inputs["ep_w3"]))
    put(b64, _OFF_B64, b64w, "ew1", a(inputs["ep_w1"]))
    put(b64, _OFF_B64, b64w, "attrT",
        np.concatenate([bbox / BBOX_MAX, dirs], axis=1).T)
    put(b64, _OFF_B64, b64w, "p0row", p0.reshape(1, 8))
    put(b64, _OFF_B64, b64w, "pw1r", np.tile(a(inputs["pr_w1"]), (7, 1)))
    mb = np.zeros((28, 56), f16)
    for jp in range(7):
        mb[jp * 4:(jp + 1) * 4, jp * 8:(jp + 1) * 8] = 1.0
    put(b64, _OFF_B64, b64w, "maskblk", mb)
    dselp = np.zeros((E, 8), f16)
    for jp in range(7):
        for i in range(N):
            dselp[jp * 8 + i, DST[i * 7 + jp]] = 1.0
    put(b64, _OFF_B64, b64w, "dselp", dselp)
    put(b64, _OFF_B64, b64w, "oneminusI",
        np.ones((8, 8), f16) - np.eye(8, dtype=f16))
    put(b64, _OFF_B64, b64w, "eye8", np.eye(8, dtype=f16))
    put(b64, _OFF_B64, b64w, "ones8", np.ones((1, 8), f16))

    def slot(img, name, data):
        r, c, w = _B64_SLOTS[name]
        data = np.asarray(data, f16).reshape(1, w)
        img[r, c:c + w] = data[0]

    b2 = a(inputs["ncp_b2"]).reshape(2, 128)
    for m in range(2):
        slot(b64, "b2m%d" % m, b2[m])
    slot(b64, "ones8_32", np.ones((1, 8), f16))
    put(b64, _OFF_B64, b64w, "eb3row", a(inputs["ep_b3"]).reshape(1, 21))
    put(b64, _OFF_B64, b64w, "eb2col", a(inputs["ep_b2"]).reshape(64, 1))
    put(w1ax, _OFF_W1AX, wax, "pb1col", a(inputs["pr_b1"]).reshape(128, 1))
    put(w1ax, _OFF_W1AX, wax, "b1cols", a(inputs["ncp_b1"]).reshape(4, 128).T)
    put(w1ax, _OFF_W1AX, wax, "eb1cols", a(inputs["ep_b1"]).reshape(2, 128).T)

    w3_full = a(inputs["ncp_w3"])
    b3_full = a(inputs["ncp_b3"])
    pw2_full = a(inputs["pr_w2"])
    pb2_full = a(inputs["pr_b2"])
    root_full = a(inputs["root"])
    bias = a(inputs["bias"]).reshape(ACT_CH)

    in_maps = []
    for j in range(N_CORES):
        cs = slice(j * CS, (j + 1) * CS)
        c4s = slice(j * C4O, (j + 1) * C4O)
        w2xj = w2x.copy()
        b64j = b64.copy()
        rootpb = np.concatenate(
            [root_full[cs], pb2_full[c4s].reshape(CS, ACT_CH)], axis=1)
        put(w2xj, _OFF_W2X, w2xw, "rootpb", _chunked(rootpb.astype(f16), 2))
        b3 = b3_full[cs].reshape(2, 128)
        for m in range(2):
            slot(b64j, "b3m%d" % m, b3[m])
        put(b64j, _OFF_B64, b64w, "biasrow4",
            bias.reshape(1, 4) if j == 0 else np.zeros((1, 4), f16))
        w3j = _chunked(np.ascontiguousarray(w3_full[:, cs]), 2)
        # pw2 image: [p, (o, q, h)] = pw2[h, (q*128+p)*4 + o]
        t = pw2_full[:, c4s].reshape(128, 2, 128, ACT_CH)   # (h, q, p, o)
        pw2pT = np.ascontiguousarray(t.transpose(2, 3, 1, 0)).reshape(128, -1)
        in_maps.append({
            "w1ax": w1ax, "w1b": w1b, "w2x": w2xj,
            "w3": np.asarray(w3j, f16),
            "pw2a": np.asarray(pw2pT[:, 0:512], f16),
            "pw2b": np.asarray(pw2pT[:, 512:1024], f16),
            "b64": b64j,
        })
    return in_maps


def kernel(**inputs):
    from concourse.bass_utils import run_bass_kernel_spmd

    if "nc" not in _NC_CACHE:
        _NC_CACHE["nc"] = build_nc()
    nc = _NC_CACHE["nc"]
    in_maps = make_in_maps(inputs)
    res = run_bass_kernel_spmd(nc, in_maps, list(range(N_CORES)))
    tot = np.zeros((8, 4), np.float32)
    for r in res.results:
        tot += np.asarray(r["outB"], np.float32)
    return tot


# revision 24
# speedup vs baseline: 1.0272x; 1.0272x over previous
"""Trainium2 Bass kernel for nn_LogicityPredictorVis.

The reference returns agg + x @ root + bias with shape [8, 4], which depends
ONLY on batch element 0 of every batched input (node_concepts[0], edge_attr[0],
batch_priorities[0]).  The B=4096 MLP sweep is dead code w.r.t. the output, so
the kernel computes just the batch-0 path.

Sharding: the NODE_CH=2048 contraction (node-MLP layer 3, the NNConv einsum,
and x @ root) is split over the 8 cores (256 channels each).  The small
replicated layers (node-MLP layers 1/2, edge MLP, pr layer 1) run on every
core.  Each core emits a partial [8,4] result; the host sums them.

The graded metric is the TimelineSim device-occupancy time of the (identical)
per-core program.  Bound by: the serial 360 GB/s DMA pipe (~5.2 us of fp16
images), the ~1.9 us issue->transfer pipe-start latency, the ~900 ns DMA
completion-semaphore propagation, the serialized cross-engine hop chain from
the last weight to the output DMA, and ~2.4 us of output-DMA + barrier
epilogue.  Design points:

* Everything DMA'd is float16 (PSUM accumulates fp32); ~1.8 MB/core, 1
  cycle/row matmuls, rel err ~1e-3 vs the 2e-2 gate.
* Eight DMAs, four per HWDGE-issuing engine (SP / Activation), interleaved so
  the serial descriptor-gen chain never starves the transfer pipe and images
  land in consumer order: w1ax, b1row, b64, w1b, w2x, w3, pw2a, pw2b.
* ALL layer biases are K=1 matmuls into the accumulation groups, fed from a
  4-ns one-row image (b1row), so every relu is a single DVE max op and the
  DVE queue (the scarce resource: each op costs ~300 ns with its ack gap)
  carries only: priority compare, 5 max ops, rhs2, 2 prod2 halves, 3 PSUM
  copies.
* The einsum weights pw2 ride last, split in two halves so the first two
  G o-slices and their prod2 multiply run under the last transfer; the final
  accumulation is one [8,4] PSUM group (bias + x@root via eye + xb via the
  complete-graph (1-I) identity + dselp-aggregated messages), one copy, one
  SP dma_start.
"""

import numpy as np

B, N = 4096, 8
C_IMG = 1024
NODE_CH = 2048
EDGE_CH = 3
ACT_CH = 4
E = N * (N - 1)
BBOX_MAX = 1024.0
N_CORES = 8
CS = NODE_CH // N_CORES        # 256 channels per core
C4O = CS * ACT_CH              # 1024 (c,o) pairs per core

_IDX = np.array([[i, j] for i in range(N) for j in range(N) if i != j],
                dtype=np.int32)
SRC = _IDX[:, 0]
DST = _IDX[:, 1]

# ---- packed fp16 input images ----
_W1AX = [("x0T", 64), ("ones128", 1), ("pb1col", 1), ("b1cols", 4),
         ("eb1cols", 2), ("ew2", 128), ("w1a", 2048)]
_W2X = [("w2", 1024), ("rootpb", 16)]
_B64 = [
    ("ew3", 21), ("dselp", 8), ("pw1r", 128), ("maskblk", 56),
    ("ew1", 256), ("attrT", 8), ("oneminusI", 8), ("eye8", 8),
    ("p0row", 8), ("ones8", 8), ("eb3row", 21), ("biasrow4", 4),
    ("eb2col", 1), ("pad", 56),
]
_B64_PARTS = {
    "ew3": 64, "dselp": 56, "pw1r": 28, "maskblk": 28, "ew1": 8,
    "attrT": 8, "oneminusI": 8, "eye8": 8, "p0row": 1, "ones8": 1,
    "eb3row": 1, "biasrow4": 1, "eb2col": 64, "pad": 1,
}
# Bias rows + ones rows ride in b64's dead rows (under the <=28-partition
# tensors), at 32-aligned partitions so K=1 matmul operands are legal.
# name -> (row, col, width); each bias matmul pairs with the ones8 copy in
# the same row (matmul requires equal base partitions).
_B64_SLOTS = {
    "b2m0": (32, 29, 128), "b2m1": (32, 157, 128),
    "b3m0": (32, 285, 128), "b3m1": (32, 413, 128),
    "ones8_32": (32, 541, 8),
}


def _offsets(specs):
    offs, off = {}, 0
    for n, c in specs:
        offs[n] = off
        off += c
    return offs, off


_OFF_W1AX, COLS_W1AX = _offsets(_W1AX)
_OFF_W2X, COLS_W2X = _offsets(_W2X)
_OFF_B64, COLS_B64 = _offsets(_B64)

_NC_CACHE = {}


def build_nc():
    """Build the per-core Bass program (identical on all cores)."""
    import concourse.bacc as bacc
    import concourse.mybir as mybir
    import concourse.tile as tile

    fp32 = mybir.dt.float32
    fp16 = mybir.dt.float16
    AF = mybir.ActivationFunctionType
    ALU = mybir.AluOpType

    nc = bacc.Bacc("TRN2", target_bir_lowering=False, debug=False)
    w1ax_d = nc.dram_tensor("w1ax", [128, COLS_W1AX], fp16,
                            kind="ExternalInput")
    w1b_d = nc.dram_tensor("w1b", [128, 2048], fp16, kind="ExternalInput")
    w2x_d = nc.dram_tensor("w2x", [128, COLS_W2X], fp16, kind="ExternalInput")
    w3_d = nc.dram_tensor("w3", [128, 512], fp16, kind="ExternalInput")
    pw2a_d = nc.dram_tensor("pw2a", [128, 512], fp16, kind="ExternalInput")
    pw2b_d = nc.dram_tensor("pw2b", [128, 512], fp16, kind="ExternalInput")
    b64_d = nc.dram_tensor("b64", [64, COLS_B64], fp16, kind="ExternalInput")
    outB_d = nc.dram_tensor("outB", [8, 4], fp32, kind="ExternalOutput")

    with tile.TileContext(nc) as tc:
        with tc.tile_pool(name="sb", bufs=1) as sb, \
             tc.tile_pool(name="ps", bufs=1, space="PSUM") as ps:

            w1ax_sb = sb.tile([128, COLS_W1AX], fp16, tag="w1ax")
            w1b_sb = sb.tile([128, 2048], fp16, tag="w1b")
            w2x_sb = sb.tile([128, COLS_W2X], fp16, tag="w2x")
            w3_sb = sb.tile([128, 512], fp16, tag="w3")
            pw2a_sb = sb.tile([128, 512], fp16, tag="pw2a")
            pw2b_sb = sb.tile([128, 512], fp16, tag="pw2b")
            b64_sb = sb.tile([64, COLS_B64], fp16, tag="b64")

            # ---- DMA issue.  SP: b64, w1b, w2x, w3, pw2a, pw2b (serial,
            # 650 ns apart); Act: w1ax.  HWDGE arrival (= transfer) order:
            # b64, w1ax, w1b, w2x, w3, pw2a, pw2b — b64 lands FIRST so the
            # edge-chain ladder (whose g2T stationary-load otherwise
            # head-blocks the in-order PE stream right when the w1b-gated
            # L1 m23 matmuls become ready) completes early.
            nc.sync.dma_start(w1ax_sb[:], w1ax_d[:])
            nc.sync.dma_start(w1b_sb[:], w1b_d[:])
            nc.sync.dma_start(w2x_sb[:], w2x_d[:])
            nc.sync.dma_start(w3_sb[:], w3_d[:])
            nc.sync.dma_start(pw2a_sb[:], pw2a_d[:])
            nc.sync.dma_start(pw2b_sb[:], pw2b_d[:])
            nc.scalar.dma_start(b64_sb[:], b64_d[:])

            def vax(name):
                off = _OFF_W1AX[name]
                cc = dict(_W1AX)[name]
                return w1ax_sb[:, off:off + cc]

            def v64(name):
                off = _OFF_B64[name]
                cc = dict(_B64)[name]
                return b64_sb[0:_B64_PARTS[name], off:off + cc]

            def vb(name):
                r, c, w = _B64_SLOTS[name]
                return b64_sb[r:r + 1, c:c + w]

            def bias_mm(out_view, slot):
                """K=1 group-stop bias matmul from the b64 row-32 slots."""
                nc.tensor.matmul(out_view, vb(slot), vb("ones8_32"),
                                 start=False, stop=True,
                                 skip_group_check=True)

            x0T_v = vax("x0T").rearrange("p (q n) -> p q n", q=8)
            w1a_v = vax("w1a").rearrange("p (m q k) -> p m q k", m=2, q=8)
            w1b_v = w1b_sb[:].rearrange("p (m q k) -> p m q k", m=2, q=8)
            ew2_v = vax("ew2").rearrange("p (q m) -> p q m", q=2)
            ones128_v = vax("ones128")
            w2_v = w2x_sb[:, 0:1024].rearrange("p (q m) -> p q m", q=4)
            rootpb_v = w2x_sb[:, _OFF_W2X["rootpb"]:].rearrange(
                "p (q m) -> p q m", q=2)
            w3_v = w3_sb[:].rearrange("p (q m) -> p q m", q=2)
            pw2a_v = pw2a_sb[:].rearrange("p (o q h) -> p o q h", o=2, q=2)
            pw2b_v = pw2b_sb[:].rearrange("p (o q h) -> p o q h", o=2, q=2)
            ew3_v, dselp_v, pw1r_v = v64("ew3"), v64("dselp"), v64("pw1r")
            maskblk_v, ew1_v, attrT_v = v64("maskblk"), v64("ew1"), v64("attrT")
            oneminusI_v, eye8_v = v64("oneminusI"), v64("eye8")
            p0row_v, ones8_v = v64("p0row"), v64("ones8")

            # fp32 scalar columns for the fused (add bias, max 0) relus
            # whose bias is per-partition (g2T, tT): DVE scalar port is fp32.
            pb1f_sb = sb.tile([128, 1], fp32, tag="pb1f")
            nc.vector.tensor_copy(pb1f_sb[:], vax("pb1col"))
            eb2f_sb = sb.tile([64, 1], fp32, tag="eb2f")
            nc.vector.tensor_copy(eb2f_sb[:], v64("eb2col"))

            # ============ compute, in dataflow order ========================
            # L1 m=0,1 (gated on w1ax; bias mm per group from b1row)
            p_h1 = ps.tile([128, 4, N], fp32, tag="ps_n", bufs=2)
            for m in range(2):
                for q in range(8):
                    nc.tensor.matmul(p_h1[:, m, :], w1a_v[:, m, q, :],
                                     x0T_v[:, q, :], start=(q == 0),
                                     stop=(q == 7), skip_group_check=True)
            # ---- edge chain (gated on b64 + b1row) ----
            p_pp = ps.tile([8, 2, 8], fp32, tag="ps_e", bufs=1)
            nc.tensor.matmul(p_pp[:, 0, :], p0row_v, ones8_v, start=True,
                             stop=True, skip_group_check=True)
            nc.tensor.matmul(p_pp[:, 1, :], ones8_v, p0row_v, start=True,
                             stop=True, skip_group_check=True)
            q4_sb = sb.tile([8, 7, 4], fp16, tag="q4")
            pc_sb = sb.tile([8, 8], fp16, tag="pc")
            nc.vector.tensor_copy(pc_sb[:], p_pp[:, 0, :])
            nc.vector.tensor_tensor(q4_sb[:, :, 3], pc_sb[:, 0:7],
                                    p_pp[:, 1, 0:7], op=ALU.is_gt)
            p_g1 = ps.tile([128, 2, N], fp32, tag="ps_e", bufs=1)
            for m in range(2):
                nc.tensor.matmul(p_g1[:, m, :],
                                 ew1_v[:, m * 128:(m + 1) * 128],
                                 attrT_v, start=True, stop=True,
                                 skip_group_check=True)
            g1T_sb = sb.tile([128, 2, N], fp16, tag="g1T")
            nc.vector.tensor_tensor(g1T_sb[:], p_g1[:],
                                    vax("eb1cols").unsqueeze(2)
                                    .broadcast_to([128, 2, N]),
                                    op=ALU.add)
            nc.vector.tensor_scalar(g1T_sb[:], g1T_sb[:], 0.0, None,
                                    op0=ALU.max)
            p_g2 = ps.tile([64, N], fp32, tag="ps_e", bufs=1)
            for q in range(2):
                nc.tensor.matmul(p_g2[:], ew2_v[:, q, :], g1T_sb[:, q, :],
                                 start=(q == 0), stop=(q == 1),
                                 skip_group_check=True)
            g2T_sb = sb.tile([64, N], fp16, tag="g2T")
            nc.vector.tensor_scalar(g2T_sb[:], p_g2[:], eb2f_sb[:], 0.0,
                                    op0=ALU.add, op1=ALU.max)
            p_ea = ps.tile([8, 21], fp32, tag="ps_e", bufs=1)
            nc.tensor.matmul(p_ea[:], g2T_sb[:], ew3_v, start=True,
                             stop=False, skip_group_check=True)
            nc.tensor.matmul(p_ea[:], ones8_v, v64("eb3row"), start=False,
                             stop=True, skip_group_check=True)
            nc.scalar.activation(q4_sb[:, :, 0:3],
                                 p_ea[:].rearrange("i (j c) -> i j c", c=3),
                                 AF.Sigmoid)
            # ---- L1 m=2,3 (gated on w1b), single wide max ----
            for m in range(2):
                for q in range(8):
                    nc.tensor.matmul(p_h1[:, m + 2, :], w1b_v[:, m, q, :],
                                     x0T_v[:, q, :], start=(q == 0),
                                     stop=(q == 7), skip_group_check=True)
            h1T_sb = sb.tile([128, 4, N], fp16, tag="h1T")
            nc.vector.tensor_tensor(h1T_sb[:], p_h1[:],
                                    vax("b1cols").unsqueeze(2)
                                    .broadcast_to([128, 4, N]),
                                    op=ALU.add)
            nc.vector.tensor_scalar(h1T_sb[:], h1T_sb[:], 0.0, None,
                                    op0=ALU.max)
            # ---- edge tail: transpose, rhs2 (PSUM-direct), t ----
            p_q4T = ps.tile([28, 8], fp16, tag="ps_e", bufs=1)
            nc.tensor.transpose(p_q4T[:],
                                q4_sb[:].rearrange("i j c -> i (j c)"),
                                eye8_v)
            rhs2_sb = sb.tile([28, E], fp16, tag="rhs2")
            nc.vector.tensor_tensor(
                rhs2_sb[:].rearrange("p (j i) -> p j i", i=8),
                p_q4T[:].unsqueeze(1).broadcast_to([28, 7, N]),
                maskblk_v.rearrange("p (j i) -> p j i", i=8),
                op=ALU.mult)
            p_t = ps.tile([128, E], fp32, tag="ps_G", bufs=2)
            nc.tensor.matmul(p_t[:], pw1r_v, rhs2_sb[:], start=True,
                             stop=True)
            tT_sb = sb.tile([128, E], fp16, tag="tT")
            nc.vector.tensor_scalar(tT_sb[:], p_t[:], pb1f_sb[:], 0.0,
                                    op0=ALU.add, op1=ALU.max)
            # ---- L2 (gated on w2x), single max ----
            p_h2 = ps.tile([128, 2, N], fp32, tag="ps_n", bufs=2)
            for m in range(2):
                for q in range(4):
                    nc.tensor.matmul(p_h2[:, m, :],
                                     w2_v[:, q, m * 128:(m + 1) * 128],
                                     h1T_sb[:, q, :], start=(q == 0),
                                     stop=False, skip_group_check=True)
                bias_mm(p_h2[:, m, :], "b2m%d" % m)
            h2T_sb = sb.tile([128, 2, N], fp16, tag="h2T")
            nc.vector.tensor_scalar(h2T_sb[:], p_h2[:], 0.0, None,
                                    op0=ALU.max)
            # ---- L3 (gated on w3) + bias mm, single sigmoid ----
            p_x = ps.tile([128, 2, N], fp32, tag="ps_n", bufs=2)
            for m in range(2):
                for q in range(2):
                    nc.tensor.matmul(p_x[:, m, :],
                                     w3_v[:, q, m * 128:(m + 1) * 128],
                                     h2T_sb[:, q, :], start=(q == 0),
                                     stop=False, skip_group_check=True)
                bias_mm(p_x[:, m, :], "b3m%d" % m)
            xT_sb = sb.tile([128, 2, N], fp16, tag="xT")
            nc.scalar.activation(xT_sb[:], p_x[:], AF.Sigmoid)
            # ---- x @ [root | pb2]; G + prod2 in pw2a / pw2b halves ----
            p_o2 = ps.tile([8, 8], fp32, tag="ps_g", bufs=3)
            for q in range(2):
                nc.tensor.matmul(p_o2[:], xT_sb[:, q, :], rootpb_v[:, q, :],
                                 start=(q == 0), stop=(q == 1),
                                 skip_group_check=True)
            p_G = ps.tile([128, 4, N], fp32, tag="ps_G", bufs=2)
            prod2_sb = sb.tile([128, 4, 7, N], fp16, tag="prod2")
            for o in range(2):
                for q in range(2):
                    nc.tensor.matmul(p_G[:, o, :], pw2a_v[:, o, q, :],
                                     xT_sb[:, q, :], start=(q == 0),
                                     stop=(q == 1), skip_group_check=True)
            nc.vector.tensor_tensor(
                prod2_sb[:, 0:2, :, :],
                tT_sb[:].rearrange("p (j i) -> p j i", i=8).unsqueeze(1)
                        .broadcast_to([128, 2, 7, N]),
                p_G[:, 0:2, :].unsqueeze(2).broadcast_to([128, 2, 7, N]),
                op=ALU.mult)
            o2_sb = sb.tile([8, 8], fp16, tag="o2")
            nc.vector.tensor_copy(o2_sb[:], p_o2[:])
            for o in range(2):
                for q in range(2):
                    nc.tensor.matmul(p_G[:, o + 2, :], pw2b_v[:, o, q, :],
                                     xT_sb[:, q, :], start=(q == 0),
                                     stop=(q == 1), skip_group_check=True)
            nc.vector.tensor_tensor(
                prod2_sb[:, 2:4, :, :],
                tT_sb[:].rearrange("p (j i) -> p j i", i=8).unsqueeze(1)
                        .broadcast_to([128, 2, 7, N]),
                p_G[:, 2:4, :].unsqueeze(2).broadcast_to([128, 2, 7, N]),
                op=ALU.mult)
            # ---- final accumulation group + per-edge sums ----
            p_s4 = ps.tile([56, 4], fp32, tag="ps_g", bufs=3)
            for o in range(4):
                nc.tensor.matmul(p_s4[:, o:o + 1],
                                 prod2_sb[:, o, :, :].rearrange(
                                     "p j i -> p (j i)"),
                                 ones128_v, start=True, stop=True,
                                 skip_group_check=True)
            s4_sb = sb.tile([56, 4], fp16, tag="s4")
            nc.vector.tensor_copy(s4_sb[:], p_s4[:])
            p_o3 = ps.tile([8, 4], fp32, tag="ps_g", bufs=3)
            nc.tensor.matmul(p_o3[:], ones8_v, v64("biasrow4"),
                             start=True, stop=False, skip_group_check=True)
            nc.tensor.matmul(p_o3[:], eye8_v, o2_sb[:, 0:4], start=False,
                             stop=False, skip_group_check=True)
            nc.tensor.matmul(p_o3[:], oneminusI_v, o2_sb[:, 4:8],
                             start=False, stop=False, skip_group_check=True)
            nc.tensor.matmul(p_o3[:], dselp_v, s4_sb[:], start=False,
                             stop=True, skip_group_check=True)
            o3_sb = sb.tile([8, 4], fp32, tag="o3")
            nc.vector.tensor_copy(o3_sb[:], p_o3[:])
            nc.sync.dma_start(outB_d[:], o3_sb[:])

    nc.compile()
    return nc


def _chunked(x, q):
    """[q*128, m] -> [128, q*m] image (partition p holds chunk-major rows)."""
    q128, m = x.shape
    assert q128 == q * 128
    return x.reshape(q, 128, m).transpose(1, 0, 2).reshape(128, q * m)


def make_in_maps(inputs):
    """Host-side sharding: build the per-core packed fp16 images (numpy)."""
    f16 = np.float16
    f32 = np.float32

    def a(x):
        return np.ascontiguousarray(np.asarray(x, dtype=f32))

    roi = a(inputs["roi_features"][0])
    bbox = a(inputs["batch_bboxes"][0])
    dirs = a(inputs["batch_directions"][0])
    p0 = a(inputs["batch_priorities"][0])

    w1ax = np.zeros((128, COLS_W1AX), f16)
    w1b = np.zeros((128, 2048), f16)
    w2x = np.zeros((128, COLS_W2X), f16)
    b64 = np.zeros((64, COLS_B64), f16)

    def put(img, offs, widths, name, data):
        off = offs[name]
        cc = widths[name]
        data = np.asarray(data, f16)
        pp = data.shape[0]
        assert data.shape == (pp, cc), (name, data.shape, cc)
        img[0:pp, off:off + cc] = data

    wax, w2xw, b64w = dict(_W1AX), dict(_W2X), dict(_B64)
    w1 = a(inputs["ncp_w1"]).reshape(8, 128, 4, 128)
    w1mqk = np.ascontiguousarray(w1.transpose(1, 2, 0, 3)).reshape(128, 4096)
    put(w1ax, _OFF_W1AX, wax, "w1a", w1mqk[:, 0:2048])
    w1b[:] = w1mqk[:, 2048:4096]
    put(w1ax, _OFF_W1AX, wax, "x0T", _chunked(a(roi.T), 8))
    put(w1ax, _OFF_W1AX, wax, "ones128", np.ones((128, 1), f16))
    put(w1ax, _OFF_W1AX, wax, "ew2", _chunked(a(inputs["ep_w2"]), 2))

    put(w2x, _OFF_W2X, w2xw, "w2", _chunked(a(inputs["ncp_w2"]), 4))

    put(b64, _OFF_B64, b64w, "ew3", a(inputs["ep_w3"]))
    put(b64, _OFF_B64, b64w, "ew1", a(inputs["ep_w1"]))
    put(b64, _OFF_B64, b64w, "attrT",
        np.concatenate([bbox / BBOX_MAX, dirs], axis=1).T)
    put(b64, _OFF_B64, b64w, "p0row", p0.reshape(1, 8))
    put(b64, _OFF_B64, b64w, "pw1r", np.tile(a(inputs["pr_w1"]), (7, 1)))
    mb = np.zeros((28, 56), f16)
    for jp in range(7):
        mb[jp * 4:(jp + 1) * 4, jp * 8:(jp + 1) * 8] = 1.0
    put(b64, _OFF_B64, b64w, "maskblk", mb)
    dselp = np.zeros((E, 8), f16)
    for jp in range(7):
        for i in range(N):
            dselp[jp * 8 + i, DST[i * 7 + jp]] = 1.0
    put(b64, _OFF_B64, b64w, "dselp", dselp)
    put(b64, _OFF_B64, b64w, "oneminusI",
        np.ones((8, 8), f16) - np.eye(8, dtype=f16))
    put(b64, _OFF_B64, b64w, "eye8", np.eye(8, dtype=f16))
    put(b64, _OFF_B64, b64w, "ones8", np.ones((1, 8), f16))

    def slot(img, name, data):
        r, c, w = _B64_SLOTS[name]
        data = np.asarray(data, f16).reshape(1, w)
        img[r, c:c + w] = data[0]

    b2 = a(inputs["ncp_b2"]).reshape(2, 128)
    for m in range(2):
        slot(b64, "b2m%d" % m, b2[m])
    slot(b64, "ones8_32", np.ones((1, 8), f16))
    put(b64, _OFF_B64, b64w, "eb3row", a(inputs["ep_b3"]).reshape(1, 21))
    put(b64, _OFF_B64, b64w, "eb2col", a(inputs["ep_b2"]).reshape(64, 1))
    put(w1ax, _OFF_W1AX, wax, "pb1col", a(inputs["pr_b1"]).reshape(128, 1))
    put(w1ax, _OFF_W1AX, wax, "b1cols", a(inputs["ncp_b1"]).reshape(4, 128).T)
    put(w1ax, _OFF_W1AX, wax, "eb1cols", a(inputs["ep_b1"]).reshape(2, 128).T)

    w3_full = a(inputs["ncp_w3"])
    b3_full = a(inputs["ncp_b3"])
    pw2_full = a(inputs["pr_w2"])
    pb2_full = a(inputs["pr_b2"])
    root_full = a(inputs["root"])
    bias = a(inputs["bias"]).reshape(ACT_CH)

    in_maps = []
    for j in range(N_CORES):
        cs = slice(j * CS, (j + 1) * CS)
        c4s = slice(j * C4O, (j + 1) * C4O)
        w2xj = w2x.copy()
        b64j = b64.copy()
        rootpb = np.concatenate(
            [root_full[cs], pb2_full[c4s].reshape(CS, ACT_CH)], axis=1)
        put(w2xj, _OFF_W2X, w2xw, "rootpb", _chunked(rootpb.astype(f16), 2))
        b3 = b3_full[cs].reshape(2, 128)
        for m in range(2):
            slot(b64j, "b3m%d" % m, b3[m])
        put(b64j, _OFF_B64, b64w, "biasrow4",
            bias.reshape(1, 4) if j == 0 else np.zeros((1, 4), f16))
        w3j = _chunked(np.ascontiguousarray(w3_full[:, cs]), 2)
        # pw2 image: [p, (o, q, h)] = pw2[h, (q*128+p)*4 + o]
        t = pw2_full[:, c4s].reshape(128, 2, 128, ACT_CH)   # (h, q, p, o)
        pw2pT = np.ascontiguousarray(t.transpose(2, 3, 1, 0)).reshape(128, -1)
        in_maps.append({
            "w1ax": w1ax, "w1b": w1b, "w2x": w2xj,
            "w3": np.asarray(w3j, f16),
            "pw2a": np.asarray(pw2pT[:, 0:512], f16),
            "pw2b": np.asarray(pw2pT[:, 512:1024], f16),
            "b64": b64j,
        })
    return in_maps


def kernel(**inputs):
    from concourse.bass_utils import run_bass_kernel_spmd

    if "nc" not in _NC_CACHE:
        _NC_CACHE["nc"] = build_nc()
    nc = _NC_CACHE["nc"]
    in_maps = make_in_maps(inputs)
    res = run_bass_kernel_spmd(nc, in_maps, list(range(N_CORES)))
    tot = np.zeros((8, 4), np.float32)
    for r in res.results:
        tot += np.asarray(r["outB"], np.float32)
    return tot


# revision 25
# speedup vs baseline: 1.0682x; 1.0399x over previous
"""Trainium2 Bass kernel for nn_LogicityPredictorVis.

The reference returns agg + x @ root + bias with shape [8, 4], which depends
ONLY on batch element 0 of every batched input (node_concepts[0], edge_attr[0],
batch_priorities[0]).  The B=4096 MLP sweep is dead code w.r.t. the output, so
the kernel computes just the batch-0 path.

Sharding: the NODE_CH=2048 contraction (node-MLP layer 3, the NNConv einsum,
and x @ root) is split over the 8 cores (256 channels each).  The small
replicated layers (node-MLP layers 1/2, edge MLP, pr layer 1) run on every
core.  Each core emits a partial [8,4] result; the host sums them.

The graded metric is the TimelineSim device-occupancy time of the (identical)
per-core program.  Bound by: the serial 360 GB/s DMA pipe (~5.2 us of fp16
images), the ~1.9 us issue->transfer pipe-start latency, the ~900 ns DMA
completion-semaphore propagation, the serialized cross-engine hop chain from
the last weight to the output DMA, and ~2.4 us of output-DMA + barrier
epilogue.  Design points:

* Everything DMA'd is float16 (PSUM accumulates fp32); ~1.8 MB/core, 1
  cycle/row matmuls, rel err ~1e-3 vs the 2e-2 gate.
* Eight DMAs, four per HWDGE-issuing engine (SP / Activation), interleaved so
  the serial descriptor-gen chain never starves the transfer pipe and images
  land in consumer order: w1ax, b1row, b64, w1b, w2x, w3, pw2a, pw2b.
* ALL layer biases are K=1 matmuls into the accumulation groups, fed from a
  4-ns one-row image (b1row), so every relu is a single DVE max op and the
  DVE queue (the scarce resource: each op costs ~300 ns with its ack gap)
  carries only: priority compare, 5 max ops, rhs2, 2 prod2 halves, 3 PSUM
  copies.
* The einsum weights pw2 ride last, split in two halves so the first two
  G o-slices and their prod2 multiply run under the last transfer; the final
  accumulation is one [8,4] PSUM group (bias + x@root via eye + xb via the
  complete-graph (1-I) identity + dselp-aggregated messages), one copy, one
  SP dma_start.
"""

import numpy as np

B, N = 4096, 8
C_IMG = 1024
NODE_CH = 2048
EDGE_CH = 3
ACT_CH = 4
E = N * (N - 1)
BBOX_MAX = 1024.0
N_CORES = 8
CS = NODE_CH // N_CORES        # 256 channels per core
C4O = CS * ACT_CH              # 1024 (c,o) pairs per core

_IDX = np.array([[i, j] for i in range(N) for j in range(N) if i != j],
                dtype=np.int32)
SRC = _IDX[:, 0]
DST = _IDX[:, 1]

# ---- packed fp16 input images ----
_W1AX = [("x0T", 64), ("ones128", 1), ("pb1col", 1), ("b1cols", 4),
         ("eb1cols", 2), ("ew2", 128), ("w1a", 2048)]
_W2X = [("w2", 1024), ("rootpb", 16)]
_B64 = [
    ("ew3", 21), ("dselp", 8), ("pw1r", 128), ("maskblk", 56),
    ("ew1", 256), ("attrT", 8), ("oneminusI", 8), ("eye8", 8),
    ("p0row", 8), ("ones8", 8), ("eb3row", 21), ("biasrow4", 4),
    ("eb2col", 1), ("pad", 56),
]
_B64_PARTS = {
    "ew3": 64, "dselp": 56, "pw1r": 28, "maskblk": 28, "ew1": 8,
    "attrT": 8, "oneminusI": 8, "eye8": 8, "p0row": 1, "ones8": 1,
    "eb3row": 1, "biasrow4": 1, "eb2col": 64, "pad": 1,
}
# Bias rows + ones rows ride in b64's dead rows (under the <=28-partition
# tensors), at 32-aligned partitions so K=1 matmul operands are legal.
# name -> (row, col, width); each bias matmul pairs with the ones8 copy in
# the same row (matmul requires equal base partitions).
_B64_SLOTS = {
    "b2m0": (32, 29, 128), "b2m1": (32, 157, 128),
    "b3m0": (32, 285, 128), "b3m1": (32, 413, 128),
    "ones8_32": (32, 541, 8),
}


def _offsets(specs):
    offs, off = {}, 0
    for n, c in specs:
        offs[n] = off
        off += c
    return offs, off


_OFF_W1AX, COLS_W1AX = _offsets(_W1AX)
_OFF_W2X, COLS_W2X = _offsets(_W2X)
_OFF_B64, COLS_B64 = _offsets(_B64)

_NC_CACHE = {}


def build_nc():
    """Build the per-core Bass program (identical on all cores)."""
    import concourse.bacc as bacc
    import concourse.mybir as mybir
    import concourse.tile as tile

    fp32 = mybir.dt.float32
    fp16 = mybir.dt.float16
    AF = mybir.ActivationFunctionType
    ALU = mybir.AluOpType

    nc = bacc.Bacc("TRN2", target_bir_lowering=False, debug=False)
    w1ax_d = nc.dram_tensor("w1ax", [128, COLS_W1AX], fp16,
                            kind="ExternalInput")
    w1b_d = nc.dram_tensor("w1b", [128, 2048], fp16, kind="ExternalInput")
    w2x_d = nc.dram_tensor("w2x", [128, COLS_W2X], fp16, kind="ExternalInput")
    w3_d = nc.dram_tensor("w3", [128, 512], fp16, kind="ExternalInput")
    pw2a_d = nc.dram_tensor("pw2a", [128, 512], fp16, kind="ExternalInput")
    pw2b_d = nc.dram_tensor("pw2b", [128, 512], fp16, kind="ExternalInput")
    b64_d = nc.dram_tensor("b64", [64, COLS_B64], fp16, kind="ExternalInput")
    outB_d = nc.dram_tensor("outB", [8, 4], fp32, kind="ExternalOutput")

    with tile.TileContext(nc) as tc:
        with tc.tile_pool(name="sb", bufs=1) as sb, \
             tc.tile_pool(name="ps", bufs=1, space="PSUM") as ps:

            w1ax_sb = sb.tile([128, COLS_W1AX], fp16, tag="w1ax")
            w1b_sb = sb.tile([128, 2048], fp16, tag="w1b")
            w2x_sb = sb.tile([128, COLS_W2X], fp16, tag="w2x")
            w3_sb = sb.tile([128, 512], fp16, tag="w3")
            pw2a_sb = sb.tile([128, 512], fp16, tag="pw2a")
            pw2b_sb = sb.tile([128, 512], fp16, tag="pw2b")
            b64_sb = sb.tile([64, COLS_B64], fp16, tag="b64")

            # ---- DMA issue.  SP: b64, w1b, w2x, w3, pw2a, pw2b (serial,
            # 650 ns apart); Act: w1ax.  HWDGE arrival (= transfer) order:
            # b64, w1ax, w1b, w2x, w3, pw2a, pw2b — b64 lands FIRST so the
            # edge-chain ladder (whose g2T stationary-load otherwise
            # head-blocks the in-order PE stream right when the w1b-gated
            # L1 m23 matmuls become ready) completes early.
            nc.sync.dma_start(w1ax_sb[:], w1ax_d[:])
            nc.sync.dma_start(w1b_sb[:], w1b_d[:])
            nc.sync.dma_start(w2x_sb[:], w2x_d[:])
            nc.sync.dma_start(w3_sb[:], w3_d[:])
            nc.sync.dma_start(pw2a_sb[:], pw2a_d[:])
            nc.sync.dma_start(pw2b_sb[:], pw2b_d[:])
            nc.scalar.dma_start(b64_sb[:], b64_d[:])

            def vax(name):
                off = _OFF_W1AX[name]
                cc = dict(_W1AX)[name]
                return w1ax_sb[:, off:off + cc]

            def v64(name):
                off = _OFF_B64[name]
                cc = dict(_B64)[name]
                return b64_sb[0:_B64_PARTS[name], off:off + cc]

            def vb(name):
                r, c, w = _B64_SLOTS[name]
                return b64_sb[r:r + 1, c:c + w]

            def bias_mm(out_view, slot):
                """K=1 group-stop bias matmul from the b64 row-32 slots."""
                nc.tensor.matmul(out_view, vb(slot), vb("ones8_32"),
                                 start=False, stop=True,
                                 skip_group_check=True)

            x0T_v = vax("x0T").rearrange("p (q n) -> p q n", q=8)
            w1a_v = vax("w1a").rearrange("p (m q k) -> p m q k", m=2, q=8)
            w1b_v = w1b_sb[:].rearrange("p (m q k) -> p m q k", m=2, q=8)
            ew2_v = vax("ew2").rearrange("p (q m) -> p q m", q=2)
            ones128_v = vax("ones128")
            w2_v = w2x_sb[:, 0:1024].rearrange("p (q m) -> p q m", q=4)
            rootpb_v = w2x_sb[:, _OFF_W2X["rootpb"]:].rearrange(
                "p (q m) -> p q m", q=2)
            w3_v = w3_sb[:].rearrange("p (q m) -> p q m", q=2)
            pw2a_v = pw2a_sb[:].rearrange("p (o q h) -> p o q h", o=2, q=2)
            pw2b_v = pw2b_sb[:].rearrange("p (o q h) -> p o q h", o=2, q=2)
            ew3_v, dselp_v, pw1r_v = v64("ew3"), v64("dselp"), v64("pw1r")
            maskblk_v, ew1_v, attrT_v = v64("maskblk"), v64("ew1"), v64("attrT")
            oneminusI_v, eye8_v = v64("oneminusI"), v64("eye8")
            p0row_v, ones8_v = v64("p0row"), v64("ones8")

            # fp32 scalar columns for the fused (add bias, max 0) relus
            # whose bias is per-partition (g2T, tT): DVE scalar port is fp32.
            pb1f_sb = sb.tile([128, 1], fp32, tag="pb1f")
            nc.vector.tensor_copy(pb1f_sb[:], vax("pb1col"))
            eb2f_sb = sb.tile([64, 1], fp32, tag="eb2f")
            nc.vector.tensor_copy(eb2f_sb[:], v64("eb2col"))

            # ============ compute, in dataflow order ========================
            # L1 m=0,1 (gated on w1ax; bias mm per group from b1row)
            p_h1 = ps.tile([128, 4, N], fp32, tag="ps_n", bufs=2)
            for m in range(2):
                for q in range(8):
                    nc.tensor.matmul(p_h1[:, m, :], w1a_v[:, m, q, :],
                                     x0T_v[:, q, :], start=(q == 0),
                                     stop=(q == 7), skip_group_check=True)
            # ---- edge chain (gated on b64 + b1row) ----
            p_pp = ps.tile([8, 2, 8], fp32, tag="ps_e", bufs=1)
            nc.tensor.matmul(p_pp[:, 0, :], p0row_v, ones8_v, start=True,
                             stop=True, skip_group_check=True)
            nc.tensor.matmul(p_pp[:, 1, :], ones8_v, p0row_v, start=True,
                             stop=True, skip_group_check=True)
            q4_sb = sb.tile([8, 7, 4], fp16, tag="q4")
            pc_sb = sb.tile([8, 8], fp16, tag="pc")
            nc.vector.tensor_copy(pc_sb[:], p_pp[:, 0, :])
            nc.vector.tensor_tensor(q4_sb[:, :, 3], pc_sb[:, 0:7],
                                    p_pp[:, 1, 0:7], op=ALU.is_gt)
            p_g1 = ps.tile([128, 2, N], fp32, tag="ps_e", bufs=1)
            for m in range(2):
                nc.tensor.matmul(p_g1[:, m, :],
                                 ew1_v[:, m * 128:(m + 1) * 128],
                                 attrT_v, start=True, stop=True,
                                 skip_group_check=True)
            g1T_sb = sb.tile([128, 2, N], fp16, tag="g1T")
            nc.vector.tensor_tensor(g1T_sb[:], p_g1[:],
                                    vax("eb1cols").unsqueeze(2)
                                    .broadcast_to([128, 2, N]),
                                    op=ALU.add)
            nc.vector.tensor_scalar(g1T_sb[:], g1T_sb[:], 0.0, None,
                                    op0=ALU.max)
            p_g2 = ps.tile([64, N], fp32, tag="ps_e", bufs=1)
            for q in range(2):
                nc.tensor.matmul(p_g2[:], ew2_v[:, q, :], g1T_sb[:, q, :],
                                 start=(q == 0), stop=(q == 1),
                                 skip_group_check=True)
            g2T_sb = sb.tile([64, N], fp16, tag="g2T")
            nc.vector.tensor_scalar(g2T_sb[:], p_g2[:], eb2f_sb[:], 0.0,
                                    op0=ALU.add, op1=ALU.max)
            p_ea = ps.tile([8, 21], fp32, tag="ps_e", bufs=1)
            nc.tensor.matmul(p_ea[:], g2T_sb[:], ew3_v, start=True,
                             stop=False, skip_group_check=True)
            nc.tensor.matmul(p_ea[:], ones8_v, v64("eb3row"), start=False,
                             stop=True, skip_group_check=True)
            nc.scalar.activation(q4_sb[:, :, 0:3],
                                 p_ea[:].rearrange("i (j c) -> i j c", c=3),
                                 AF.Sigmoid)
            # ---- L1 m=2,3 (gated on w1b), single wide max ----
            for m in range(2):
                for q in range(8):
                    nc.tensor.matmul(p_h1[:, m + 2, :], w1b_v[:, m, q, :],
                                     x0T_v[:, q, :], start=(q == 0),
                                     stop=(q == 7), skip_group_check=True)
            h1T_sb = sb.tile([128, 4, N], fp16, tag="h1T")
            nc.vector.tensor_tensor(h1T_sb[:], p_h1[:],
                                    vax("b1cols").unsqueeze(2)
                                    .broadcast_to([128, 4, N]),
                                    op=ALU.add)
            nc.vector.tensor_scalar(h1T_sb[:], h1T_sb[:], 0.0, None,
                                    op0=ALU.max)
            # ---- edge tail: transpose, rhs2 (PSUM-direct), t ----
            p_q4T = ps.tile([28, 8], fp16, tag="ps_e", bufs=1)
            nc.tensor.transpose(p_q4T[:],
                                q4_sb[:].rearrange("i j c -> i (j c)"),
                                eye8_v)
            rhs2_sb = sb.tile([28, E], fp16, tag="rhs2")
            nc.vector.tensor_tensor(
                rhs2_sb[:].rearrange("p (j i) -> p j i", i=8),
                p_q4T[:].unsqueeze(1).broadcast_to([28, 7, N]),
                maskblk_v.rearrange("p (j i) -> p j i", i=8),
                op=ALU.mult)
            p_t = ps.tile([128, E], fp32, tag="ps_G", bufs=2)
            nc.tensor.matmul(p_t[:], pw1r_v, rhs2_sb[:], start=True,
                             stop=True)
            tT_sb = sb.tile([128, E], fp16, tag="tT")
            nc.vector.tensor_scalar(tT_sb[:], p_t[:], pb1f_sb[:], 0.0,
                                    op0=ALU.add, op1=ALU.max)
            # ---- L2 (gated on w2x), single max ----
            p_h2 = ps.tile([128, 2, N], fp32, tag="ps_n", bufs=2)
            for m in range(2):
                for q in range(4):
                    nc.tensor.matmul(p_h2[:, m, :],
                                     w2_v[:, q, m * 128:(m + 1) * 128],
                                     h1T_sb[:, q, :], start=(q == 0),
                                     stop=False, skip_group_check=True)
                bias_mm(p_h2[:, m, :], "b2m%d" % m)
            h2T_sb = sb.tile([128, 2, N], fp16, tag="h2T")
            nc.vector.tensor_scalar(h2T_sb[:], p_h2[:], 0.0, None,
                                    op0=ALU.max)
            # ---- L3 (gated on w3) + bias mm, single sigmoid ----
            p_x = ps.tile([128, 2, N], fp32, tag="ps_n", bufs=2)
            for m in range(2):
                for q in range(2):
                    nc.tensor.matmul(p_x[:, m, :],
                                     w3_v[:, q, m * 128:(m + 1) * 128],
                                     h2T_sb[:, q, :], start=(q == 0),
                                     stop=False, skip_group_check=True)
                bias_mm(p_x[:, m, :], "b3m%d" % m)
            xT_sb = sb.tile([128, 2, N], fp16, tag="xT")
            nc.scalar.activation(xT_sb[:], p_x[:], AF.Sigmoid)
            # ---- x @ [root | pb2]; G + prod2 in pw2a / pw2b halves ----
            p_o2 = ps.tile([8, 8], fp32, tag="ps_g", bufs=3)
            for q in range(2):
                nc.tensor.matmul(p_o2[:], xT_sb[:, q, :], rootpb_v[:, q, :],
                                 start=(q == 0), stop=(q == 1),
                                 skip_group_check=True)
            p_G = ps.tile([128, 4, N], fp32, tag="ps_G", bufs=2)
            prod2_sb = sb.tile([128, 4, 7, N], fp16, tag="prod2")
            for o in range(2):
                for q in range(2):
                    nc.tensor.matmul(p_G[:, o, :], pw2a_v[:, o, q, :],
                                     xT_sb[:, q, :], start=(q == 0),
                                     stop=(q == 1), skip_group_check=True)
            for o in range(2):
                for q in range(2):
                    nc.tensor.matmul(p_G[:, o + 2, :], pw2b_v[:, o, q, :],
                                     xT_sb[:, q, :], start=(q == 0),
                                     stop=(q == 1), skip_group_check=True)
            nc.vector.tensor_tensor(
                prod2_sb[:],
                tT_sb[:].rearrange("p (j i) -> p j i", i=8).unsqueeze(1)
                        .broadcast_to([128, 4, 7, N]),
                p_G[:].unsqueeze(2).broadcast_to([128, 4, 7, N]),
                op=ALU.mult)
            o2_sb = sb.tile([8, 8], fp16, tag="o2")
            nc.vector.tensor_copy(o2_sb[:], p_o2[:])
            # ---- final accumulation group + per-edge sums ----
            p_s4 = ps.tile([56, 4], fp32, tag="ps_g", bufs=3)
            for o in range(4):
                nc.tensor.matmul(p_s4[:, o:o + 1],
                                 prod2_sb[:, o, :, :].rearrange(
                                     "p j i -> p (j i)"),
                                 ones128_v, start=True, stop=True,
                                 skip_group_check=True)
            s4_sb = sb.tile([56, 4], fp16, tag="s4")
            nc.vector.tensor_copy(s4_sb[:], p_s4[:])
            p_o3 = ps.tile([8, 4], fp32, tag="ps_g", bufs=3)
            nc.tensor.matmul(p_o3[:], ones8_v, v64("biasrow4"),
                             start=True, stop=False, skip_group_check=True)
            nc.tensor.matmul(p_o3[:], eye8_v, o2_sb[:, 0:4], start=False,
                             stop=False, skip_group_check=True)
            nc.tensor.matmul(p_o3[:], oneminusI_v, o2_sb[:, 4:8],
                             start=False, stop=False, skip_group_check=True)
            nc.tensor.matmul(p_o3[:], dselp_v, s4_sb[:], start=False,
                             stop=True, skip_group_check=True)
            o3_sb = sb.tile([8, 4], fp32, tag="o3")
            nc.vector.tensor_copy(o3_sb[:], p_o3[:])
            nc.sync.dma_start(outB_d[:], o3_sb[:])

    nc.compile()
    return nc


def _chunked(x, q):
    """[q*128, m] -> [128, q*m] image (partition p holds chunk-major rows)."""
    q128, m = x.shape
    assert q128 == q * 128
    return x.reshape(q, 128, m).transpose(1, 0, 2).reshape(128, q * m)


def make_in_maps(inputs):
    """Host-side sharding: build the per-core packed fp16 images (numpy)."""
    f16 = np.float16
    f32 = np.float32

    def a(x):
        return np.ascontiguousarray(np.asarray(x, dtype=f32))

    roi = a(inputs["roi_features"][0])
    bbox = a(inputs["batch_bboxes"][0])
    dirs = a(inputs["batch_directions"][0])
    p0 = a(inputs["batch_priorities"][0])

    w1ax = np.zeros((128, COLS_W1AX), f16)
    w1b = np.zeros((128, 2048), f16)
    w2x = np.zeros((128, COLS_W2X), f16)
    b64 = np.zeros((64, COLS_B64), f16)

    def put(img, offs, widths, name, data):
        off = offs[name]
        cc = widths[name]
        data = np.asarray(data, f16)
        pp = data.shape[0]
        assert data.shape == (pp, cc), (name, data.shape, cc)
        img[0:pp, off:off + cc] = data

    wax, w2xw, b64w = dict(_W1AX), dict(_W2X), dict(_B64)
    w1 = a(inputs["ncp_w1"]).reshape(8, 128, 4, 128)
    w1mqk = np.ascontiguousarray(w1.transpose(1, 2, 0, 3)).reshape(128, 4096)
    put(w1ax, _OFF_W1AX, wax, "w1a", w1mqk[:, 0:2048])
    w1b[:] = w1mqk[:, 2048:4096]
    put(w1ax, _OFF_W1AX, wax, "x0T", _chunked(a(roi.T), 8))
    put(w1ax, _OFF_W1AX, wax, "ones128", np.ones((128, 1), f16))
    put(w1ax, _OFF_W1AX, wax, "ew2", _chunked(a(inputs["ep_w2"]), 2))

    put(w2x, _OFF_W2X, w2xw, "w2", _chunked(a(inputs["ncp_w2"]), 4))

    put(b64, _OFF_B64, b64w, "ew3", a(inputs["ep_w3"]))
    put(b64, _OFF_B64, b64w, "ew1", a(inputs["ep_w1"]))
    put(b64, _OFF_B64, b64w, "attrT",
        np.concatenate([bbox / BBOX_MAX, dirs], axis=1).T)
    put(b64, _OFF_B64, b64w, "p0row", p0.reshape(1, 8))
    put(b64, _OFF_B64, b64w, "pw1r", np.tile(a(inputs["pr_w1"]), (7, 1)))
    mb = np.zeros((28, 56), f16)
    for jp in range(7):
        mb[jp * 4:(jp + 1) * 4, jp * 8:(jp + 1) * 8] = 1.0
    put(b64, _OFF_B64, b64w, "maskblk", mb)
    dselp = np.zeros((E, 8), f16)
    for jp in range(7):
        for i in range(N):
            dselp[jp * 8 + i, DST[i * 7 + jp]] = 1.0
    put(b64, _OFF_B64, b64w, "dselp", dselp)
    put(b64, _OFF_B64, b64w, "oneminusI",
        np.ones((8, 8), f16) - np.eye(8, dtype=f16))
    put(b64, _OFF_B64, b64w, "eye8", np.eye(8, dtype=f16))
    put(b64, _OFF_B64, b64w, "ones8", np.ones((1, 8), f16))

    def slot(img, name, data):
        r, c, w = _B64_SLOTS[name]
        data = np.asarray(data, f16).reshape(1, w)
        img[r, c:c + w] = data[0]

    b2 = a(inputs["ncp_b2"]).reshape(2, 128)
    for m in range(2):
        slot(b64, "b2m%d" % m, b2[m])
    slot(b64, "ones8_32", np.ones((1, 8), f16))
    put(b64, _OFF_B64, b64w, "eb3row", a(inputs["ep_b3"]).reshape(1, 21))
    put(b64, _OFF_B64, b64w, "eb2col", a(inputs["ep_b2"]).reshape(64, 1))
    put(w1ax, _OFF_W1AX, wax, "pb1col", a(inputs["pr_b1"]).reshape(128, 1))
    put(w1ax, _OFF_W1AX, wax, "b1cols", a(inputs["ncp_b1"]).reshape(4, 128).T)
    put(w1ax, _OFF_W1AX, wax, "eb1cols", a(inputs["ep_b1"]).reshape(2, 128).T)

    w3_full = a(inputs["ncp_w3"])
    b3_full = a(inputs["ncp_b3"])
    pw2_full = a(inputs["pr_w2"])
    pb2_full = a(inputs["pr_b2"])
    root_full = a(inputs["root"])
    bias = a(inputs["bias"]).reshape(ACT_CH)

    in_maps = []
    for j in range(N_CORES):
        cs = slice(j * CS, (j + 1) * CS)
        c4s = slice(j * C4O, (j + 1) * C4O)
        w2xj = w2x.copy()
        b64j = b64.copy()
        rootpb = np.concatenate(
            [root_full[cs], pb2_full[c4s].reshape(CS, ACT_CH)], axis=1)
        put(w2xj, _OFF_W2X, w2xw, "rootpb", _chunked(rootpb.astype(f16), 2))
        b3 = b3_full[cs].reshape(2, 128)
        for m in range(2):
            slot(b64j, "b3m%d" % m, b3[m])
        put(b64j, _OFF_B64, b64w, "biasrow4",
            bias.reshape(1, 4) if j == 0 else np.zeros((1, 4), f16))
        w3j = _chunked(np.ascontiguousarray(w3_full[:, cs]), 2)
        # pw2 image: [p, (o, q, h)] = pw2[h, (q*128+p)*4 + o]
        t = pw2_full[:, c4s].reshape(128, 2, 128, ACT_CH)   # (h, q, p, o)
        pw2pT = np.ascontiguousarray(t.transpose(2, 3, 1, 0)).reshape(128, -1)
        in_maps.append({
            "w1ax": w1ax, "w1b": w1b, "w2x": w2xj,
            "w3": np.asarray(w3j, f16),
            "pw2a": np.asarray(pw2pT[:, 0:512], f16),
            "pw2b": np.asarray(pw2pT[:, 512:1024], f16),
            "b64": b64j,
        })
    return in_maps


def kernel(**inputs):
    from concourse.bass_utils import run_bass_kernel_spmd

    if "nc" not in _NC_CACHE:
        _NC_CACHE["nc"] = build_nc()
    nc = _NC_CACHE["nc"]
    in_maps = make_in_maps(inputs)
    res = run_bass_kernel_spmd(nc, in_maps, list(range(N_CORES)))
    tot = np.zeros((8, 4), np.float32)
    for r in res.results:
        tot += np.asarray(r["outB"], np.float32)
    return tot
